# revision 35
# baseline (speedup 1.0000x reference)
"""CosineAttention Trainium2 Bass kernel.

Computes softmax(cos_sim(keys[b,l,:], query[b,:]) masked) over l, for
B=64, L=4096, D=1024, sharded batch-parallel over 8 NeuronCores
(8 batches per core, 128 MiB of keys per core -> memory bound).

Math per (b, l):
    dot[l]  = sum_d keys[b,l,d] * q[b,d]
    ssq[l]  = sum_d keys[b,l,d]^2
    score   = dot / (max(sqrt(ssq),eps) * max(||q||,eps)) + (mask-1)*1e30
    out     = exp(score) / sum_l exp(score)   (scores in [-1,1]: no max-sub)

Engine plan per core (keys stream at the HBM roofline, compute hides
underneath):
  - DMA  : gpsimd (SWDGE) streams keys in 4 MiB f32 reads, casting to
           bf16 on the fly (2 MiB SBUF tiles); rel err ~4e-4 stays well
           inside the 2e-2 gate while SBUF write pressure halves and
           prefetch depth doubles.
  - DVE  : fused mult+accum (scalar_tensor_tensor) -> dot per l
  - ACT  : Square with accum_out -> ssq per l (Square lives in every ACT
           table set, so the main loop never reloads tables)
  - PE   : ones-matmul for the cross-partition softmax denominator
Softmax runs as one batched tail (2 ACT table loads total instead of 2
per batch).  L is laid out interleaved: l = p*T + t (p = partition,
T = L/128), so keys loads and the output store are contiguous per
partition.
"""

import numpy as np

import concourse.bass as bass
import concourse.tile as tile
from concourse import bacc, mybir

P = 128          # SBUF partitions
B = 64           # full batch
L = 4096
D = 1024
N_CORES = 8
BPC = B // N_CORES   # batches per core
CJ = 8               # l-tiles per DMA chunk (4 MiB f32 reads)

F32 = mybir.dt.float32
BF16 = mybir.dt.bfloat16
U8 = mybir.dt.uint8
Alu = mybir.AluOpType
Act = mybir.ActivationFunctionType

EPS = 1e-12
NEG_BIG = 1.0e30


def build_nc(bpc=BPC, l_dim=L, d=D, cj=CJ, n_cores=N_CORES, reps=1,
             variant="full", kbufs=6, dve_ssq=0, dve_ssq_alt=1, pool_ssq=0,
             pool_dot=0, pool_dot_alt=0, kdt="bf16", loop_n=0, epi="tail"):
    do_dve = variant in ("full", "dma_dve")
    do_act = variant in ("full", "dma_act")
    KDT = BF16 if kdt == "bf16" else F32
    t_cols = l_dim // P       # score columns per partition
    nch = t_cols // cj        # chunks per batch
    cols = bpc * t_cols
    assert t_cols * P == l_dim and nch * cj == t_cols

    nc = bacc.Bacc(
        "TRN2",
        target_bir_lowering=False,
        debug=False,
        enable_asserts=False,
        num_devices=n_cores,
    )

    q_t = nc.dram_tensor("q", [bpc, d], F32, kind="ExternalInput")
    keys_t = nc.dram_tensor("keys", [bpc, l_dim, d], F32, kind="ExternalInput")
    mask_t = nc.dram_tensor("mask", [bpc, l_dim], U8, kind="ExternalInput")
    out_t = nc.dram_tensor("out", [bpc, l_dim], F32, kind="ExternalOutput")

    q_ap = q_t.ap()
    keys_ap = keys_t.ap()
    mask_ap = mask_t.ap()
    out_ap = out_t.ap()

    with tile.TileContext(nc) as tc:
        with (
            tc.tile_pool(name="kpool", bufs=kbufs) as kpool,
            tc.tile_pool(name="singles", bufs=1) as singles,
            tc.tile_pool(name="ascr", bufs=2) as ascr,
            tc.tile_pool(name="vscr", bufs=2) as vscr,
            tc.tile_pool(name="psum", bufs=2, space="PSUM") as psum,
        ):
            # --- persistent tiles ---
            qrep = singles.tile([P, bpc, d], KDT)        # q replicated to all partitions
            maskf = singles.tile([P, cols], F32)         # mask -> additive bias
            qss = singles.tile([P, bpc], F32)            # per-batch 1/||q||
            ones = singles.tile([P, P], F32)             # cross-partition sum matmul
            negbig = singles.tile([P, 1], F32)           # bias tile for mask rescale

            nc.vector.memset(ones, 1.0)
            nc.vector.memset(negbig, -NEG_BIG)

            def emit_qrep(b):
                # Broadcast q[b] to all 128 partitions during the DMA
                # (partition step 0), casting to the compute dtype on the
                # fly.  Piecewise per batch: batch 0 goes first so the
                # first dots aren't stuck behind a 4 MiB broadcast.
                q_bcast = bass.AP(
                    tensor=q_ap.tensor,
                    offset=q_ap.offset + b * d,
                    ap=[[0, P], [1, d]],
                )
                nc.gpsimd.dma_start(out=qrep[:, b, :], in_=q_bcast)

            def emit_preamble():
                for b in range(1, bpc):
                    emit_qrep(b)
                # Mask: u8 -> f32 cast during DMA.  DRAM layout per batch is
                # [P, t_cols] with l = p*t_cols + t.
                mask_v = mask_ap.rearrange("b (p t) -> p b t", p=P)
                nc.gpsimd.dma_start(
                    out=maskf[:].rearrange("p (b t) -> p b t", b=bpc), in_=mask_v
                )

                # (mask -> additive-bias rescale is deferred to the tail so
                # ACT's in-order queue isn't blocked on the mask DMA here)

                # (q norms are distributed across the first steady-state
                # chunks on Pool -- see emit_qnorm; sqrt/recip in the tail)

            def emit_qnorm(b):
                # q norm for one batch (DVE; walrus only lowers tensor ops
                # on DVE/ACT -- Pool compute fails codegen)
                s = vscr.tile([P, d], KDT, tag="vout")
                nc.vector.scalar_tensor_tensor(
                    out=s, in0=qrep[:, b, :], scalar=1.0, in1=qrep[:, b, :],
                    op0=Alu.mult, op1=Alu.mult,
                    accum_out=qss[:, b : b + 1],
                )

            import contextlib

            if loop_n or reps > 1:
                # bench-loop configs: preamble once, incl. qss/mask processing
                # (the tail would otherwise redo these in-place ops every rep)
                emit_qrep(0)
                emit_preamble()
                for b in range(bpc):
                    emit_qnorm(b)
                nc.scalar.activation(out=qss, in_=qss, func=Act.Sqrt)
                nc.vector.tensor_scalar_max(qss, qss, EPS)
                nc.vector.reciprocal(qss, qss)
                nc.scalar.activation(out=maskf, in_=maskf, func=Act.Identity,
                                     bias=negbig[:, 0:1], scale=NEG_BIG)

            loop_cm = tc.For_i(0, loop_n, 1) if loop_n else contextlib.nullcontext()
            with loop_cm:
              for _rep in range(reps):
                # per-rep accumulators (bufs=1 tags -> reps serialize on slots)
                dots = singles.tile([P, cols], F32, tag="dots")
                ssqs = singles.tile([P, cols], F32, tag="ssqs")
                if not do_dve:
                    nc.vector.memset(dots, 0.0)
                if not do_act:
                    nc.vector.memset(ssqs, 1.0)

                # --- main loop: stream keys, fused dot + ssq reductions.
                # DMA emission is software-pipelined `pf` chunks ahead of
                # compute in program order: the Pool sequencer is in-order,
                # so a Pool compute op waiting on chunk data must never sit
                # in front of the SWDGE descriptor-gen for a later chunk.
                # chunk list: (batch, first t-column, n l-tiles).  The first
                # chunk is split into small sub-chunks so the first compute
                # ops aren't stuck behind a full 4 MiB DMA at startup.
                chunks = []
                for b in range(bpc):
                    t0 = 0
                    if b == 0 and cj >= 4:
                        for _ in range(4):
                            chunks.append((b, t0, cj // 4))
                            t0 += cj // 4
                    while t0 < t_cols:
                        chunks.append((b, t0, cj))
                        t0 += cj
                kvs = [
                    keys_ap[b].rearrange("(p t) d -> p (t d)", p=P)
                    for b in range(bpc)
                ]
                tiles = {}
                pf = max(1, kbufs - 2)

                def emit_dma(i):
                    b, t0, nj = chunks[i]
                    kt = kpool.tile([P, nj, d], KDT, tag="kt")
                    kt_flat = kt[:].rearrange("p c d -> p (c d)")
                    src = kvs[b][:, t0 * d : (t0 + nj) * d]
                    if KDT == F32:
                        nc.sync.dma_start(out=kt_flat, in_=src)
                    else:
                        # SWDGE path: f32 -> bf16 cast inside the DMA
                        nc.gpsimd.dma_start(out=kt_flat, in_=src)
                    tiles[i] = kt

                def emit_compute(i):
                    b, t0, nj = chunks[i]
                    kt = tiles.pop(i)
                    pool_dot_i = pool_dot + (pool_dot_alt if i % 2 == 0 else 0)
                    dve_ssq_i = dve_ssq + (dve_ssq_alt if i % 2 == 0 else 0)
                    for j_ in range(nj):
                        idx = b * t_cols + t0 + j_
                        j = (t0 + j_) % cj   # engine split by absolute period
                        if do_dve:
                            if j >= cj - pool_dot_i:
                                pout = vscr.tile([P, d], KDT, tag="pout")
                                nc.gpsimd.scalar_tensor_tensor(
                                    out=pout,
                                    in0=kt[:, j_, :],
                                    scalar=1.0,
                                    in1=qrep[:, b, :],
                                    op0=Alu.mult,
                                    op1=Alu.mult,
                                    accum_out=dots[:, idx : idx + 1],
                                )
                            else:
                                vout = vscr.tile([P, d], KDT, tag="vout")
                                nc.vector.scalar_tensor_tensor(
                                    out=vout,
                                    in0=kt[:, j_, :],
                                    scalar=1.0,
                                    in1=qrep[:, b, :],
                                    op0=Alu.mult,
                                    op1=Alu.mult,
                                    accum_out=dots[:, idx : idx + 1],
                                )
                        if do_act:
                            if j < pool_ssq:
                                pout2 = vscr.tile([P, d], KDT, tag="pout2")
                                nc.gpsimd.scalar_tensor_tensor(
                                    out=pout2,
                                    in0=kt[:, j_, :],
                                    scalar=1.0,
                                    in1=kt[:, j_, :],
                                    op0=Alu.mult,
                                    op1=Alu.mult,
                                    accum_out=ssqs[:, idx : idx + 1],
                                )
                            elif do_dve and j < pool_ssq + dve_ssq_i:
                                vout2 = vscr.tile([P, d], KDT, tag="vout2")
                                nc.vector.scalar_tensor_tensor(
                                    out=vout2,
                                    in0=kt[:, j_, :],
                                    scalar=1.0,
                                    in1=kt[:, j_, :],
                                    op0=Alu.mult,
                                    op1=Alu.mult,
                                    accum_out=ssqs[:, idx : idx + 1],
                                )
                            else:
                                aout = ascr.tile([P, d], KDT, tag="aout")
                                nc.scalar.activation(
                                    out=aout,
                                    in_=kt[:, j_, :],
                                    func=Act.Square,
                                    accum_out=ssqs[:, idx : idx + 1],
                                )

                single = _rep == 0 and not loop_n and reps == 1
                for i in range(len(chunks)):
                    if i == 0 and single:
                        emit_qrep(0)
                    emit_dma(i)
                    if i == min(1, pf - 1) and single:
                        emit_preamble()
                    if i >= pf:
                        emit_compute(i - pf)
                        if single and 1 <= (i - pf) <= bpc:
                            emit_qnorm(i - pf - 1)
                for i in range(len(chunks) - pf, len(chunks)):
                    emit_compute(i)

                # --- batched tail: normalize, mask, softmax, store ---
                den = singles.tile([P, bpc], F32, tag="den")
                if _rep == 0 and not loop_n and reps == 1:
                    nc.scalar.activation(out=qss, in_=qss, func=Act.Sqrt)
                    nc.vector.tensor_scalar_max(qss, qss, EPS)
                    nc.vector.reciprocal(qss, qss)    # 1/||q|| per batch
                nc.scalar.activation(out=ssqs, in_=ssqs, func=Act.Sqrt)  # ||k||
                if not loop_n and reps == 1:
                    # mask -> additive bias {0, -1e30} (deferred from preamble)
                    nc.scalar.activation(out=maskf, in_=maskf, func=Act.Identity,
                                         bias=negbig[:, 0:1], scale=NEG_BIG)
                nc.vector.tensor_scalar_max(ssqs, ssqs, EPS)
                nc.vector.reciprocal(ssqs, ssqs)                         # 1/||k||
                for b in range(bpc):
                    sl = slice(b * t_cols, (b + 1) * t_cols)
                    # fold 1/||q|| into 1/||k||
                    nc.vector.tensor_scalar_mul(ssqs[:, sl], ssqs[:, sl],
                                                qss[:, b : b + 1])
                nc.vector.tensor_mul(dots, dots, ssqs)     # cos scores
                nc.vector.tensor_add(dots, dots, maskf)    # mask bias
                nc.scalar.activation(out=dots, in_=dots, func=Act.Exp)
                # denominator: ones.T @ E sums across partitions; then
                # segmented-reduce the t_cols columns per batch; every
                # partition ends up with the full per-batch sum.
                mm = psum.tile([P, cols], F32, tag="mm")
                nc.tensor.matmul(out=mm, lhsT=ones, rhs=dots,
                                 start=True, stop=True)
                nc.vector.tensor_reduce(
                    out=den,
                    in_=mm[:].rearrange("p (b t) -> p b t", b=bpc),
                    axis=mybir.AxisListType.X,
                    op=Alu.add,
                )
                nc.vector.reciprocal(den, den)
                for b in range(bpc):
                    sl = slice(b * t_cols, (b + 1) * t_cols)
                    nc.vector.tensor_scalar_mul(dots[:, sl], dots[:, sl],
                                                den[:, b : b + 1])
                out_v = out_ap.rearrange("b (p t) -> p b t", p=P)
                nc.sync.dma_start(
                    out=out_v,
                    in_=dots[:].rearrange("p (b t) -> p b t", b=bpc),
                )

    nc.compile()
    return nc


_NC_CACHE = None


def _get_nc():
    global _NC_CACHE
    if _NC_CACHE is None:
        _NC_CACHE = build_nc()
    return _NC_CACHE


def kernel(query: np.ndarray, keys: np.ndarray, mask: np.ndarray) -> np.ndarray:
    assert query.shape == (B, D) and keys.shape == (B, L, D) and mask.shape == (B, L)
    from concourse.bass_utils import run_bass_kernel_spmd

    nc = _get_nc()
    mask_u8 = np.ascontiguousarray(mask).view(np.uint8)
    in_maps = []
    for i in range(N_CORES):
        sl = slice(i * BPC, (i + 1) * BPC)
        in_maps.append(
            {
                "q": np.ascontiguousarray(query[sl], dtype=np.float32),
                "keys": np.ascontiguousarray(keys[sl], dtype=np.float32),
                "mask": np.ascontiguousarray(mask_u8[sl]),
            }
        )
    res = run_bass_kernel_spmd(nc, in_maps, core_ids=list(range(N_CORES)))
    out = np.concatenate([r["out"] for r in res.results], axis=0)
    return out.astype(np.float32, copy=False)


# revision 38
# speedup vs baseline: 1.0181x; 1.0181x over previous
"""CosineAttention Trainium2 Bass kernel.

Computes softmax(cos_sim(keys[b,l,:], query[b,:]) masked) over l, for
B=64, L=4096, D=1024, sharded batch-parallel over 8 NeuronCores
(8 batches per core, 128 MiB of keys per core -> memory bound).

Math per (b, l):
    dot[l]  = sum_d keys[b,l,d] * q[b,d]
    ssq[l]  = sum_d keys[b,l,d]^2
    score   = dot / (max(sqrt(ssq),eps) * max(||q||,eps)) + (mask-1)*1e30
    out     = exp(score) / sum_l exp(score)   (scores in [-1,1]: no max-sub)

Engine plan per core (keys stream at the HBM roofline, compute hides
underneath):
  - DMA  : gpsimd (SWDGE) streams keys in 4 MiB f32 reads, casting to
           bf16 on the fly (2 MiB SBUF tiles); rel err ~4e-4 stays well
           inside the 2e-2 gate while SBUF write pressure halves and
           prefetch depth doubles.
  - DVE  : fused mult+accum (scalar_tensor_tensor) -> dot per l
  - ACT  : Square with accum_out -> ssq per l (Square lives in every ACT
           table set, so the main loop never reloads tables)
  - PE   : ones-matmul for the cross-partition softmax denominator
Softmax runs as one batched tail (2 ACT table loads total instead of 2
per batch).  L is laid out interleaved: l = p*T + t (p = partition,
T = L/128), so keys loads and the output store are contiguous per
partition.
"""

import numpy as np

import concourse.bass as bass
import concourse.tile as tile
from concourse import bacc, mybir

P = 128          # SBUF partitions
B = 64           # full batch
L = 4096
D = 1024
N_CORES = 8
BPC = B // N_CORES   # batches per core
CJ = 8               # l-tiles per DMA chunk (4 MiB f32 reads)

F32 = mybir.dt.float32
BF16 = mybir.dt.bfloat16
U8 = mybir.dt.uint8
Alu = mybir.AluOpType
Act = mybir.ActivationFunctionType

EPS = 1e-12
NEG_BIG = 1.0e30


def build_nc(bpc=BPC, l_dim=L, d=D, cj=CJ, n_cores=N_CORES, reps=1,
             variant="full", kbufs=6, dve_ssq=0, dve_ssq_alt=1, dve_ssq_mod=3,
             pool_ssq=0, pool_dot=0, pool_dot_alt=0, kdt="bf16", loop_n=0,
             epi="tail"):
    do_dve = variant in ("full", "dma_dve")
    do_act = variant in ("full", "dma_act")
    KDT = BF16 if kdt == "bf16" else F32
    t_cols = l_dim // P       # score columns per partition
    nch = t_cols // cj        # chunks per batch
    cols = bpc * t_cols
    assert t_cols * P == l_dim and nch * cj == t_cols

    nc = bacc.Bacc(
        "TRN2",
        target_bir_lowering=False,
        debug=False,
        enable_asserts=False,
        num_devices=n_cores,
    )

    q_t = nc.dram_tensor("q", [bpc, d], F32, kind="ExternalInput")
    keys_t = nc.dram_tensor("keys", [bpc, l_dim, d], F32, kind="ExternalInput")
    mask_t = nc.dram_tensor("mask", [bpc, l_dim], U8, kind="ExternalInput")
    out_t = nc.dram_tensor("out", [bpc, l_dim], F32, kind="ExternalOutput")

    q_ap = q_t.ap()
    keys_ap = keys_t.ap()
    mask_ap = mask_t.ap()
    out_ap = out_t.ap()

    with tile.TileContext(nc) as tc:
        with (
            tc.tile_pool(name="kpool", bufs=kbufs) as kpool,
            tc.tile_pool(name="singles", bufs=1) as singles,
            tc.tile_pool(name="ascr", bufs=2) as ascr,
            tc.tile_pool(name="vscr", bufs=2) as vscr,
            tc.tile_pool(name="psum", bufs=2, space="PSUM") as psum,
        ):
            # --- persistent tiles ---
            qrep = singles.tile([P, bpc, d], KDT)        # q replicated to all partitions
            maskf = singles.tile([P, cols], F32)         # mask -> additive bias
            qss = singles.tile([P, bpc], F32)            # per-batch 1/||q||
            ones = singles.tile([P, P], F32)             # cross-partition sum matmul
            negbig = singles.tile([P, 1], F32)           # bias tile for mask rescale

            nc.vector.memset(ones, 1.0)
            nc.vector.memset(negbig, -NEG_BIG)

            def emit_qrep(b):
                # Broadcast q[b] to all 128 partitions during the DMA
                # (partition step 0), casting to the compute dtype on the
                # fly.  Piecewise per batch: batch 0 goes first so the
                # first dots aren't stuck behind a 4 MiB broadcast.
                q_bcast = bass.AP(
                    tensor=q_ap.tensor,
                    offset=q_ap.offset + b * d,
                    ap=[[0, P], [1, d]],
                )
                nc.gpsimd.dma_start(out=qrep[:, b, :], in_=q_bcast)

            def emit_preamble():
                for b in range(1, bpc):
                    emit_qrep(b)
                # Mask: u8 -> f32 cast during DMA.  DRAM layout per batch is
                # [P, t_cols] with l = p*t_cols + t.
                mask_v = mask_ap.rearrange("b (p t) -> p b t", p=P)
                nc.gpsimd.dma_start(
                    out=maskf[:].rearrange("p (b t) -> p b t", b=bpc), in_=mask_v
                )

                # (mask -> additive-bias rescale is deferred to the tail so
                # ACT's in-order queue isn't blocked on the mask DMA here)

                # (q norms are distributed across the first steady-state
                # chunks on Pool -- see emit_qnorm; sqrt/recip in the tail)

            def emit_qnorm(b):
                # q norm for one batch (DVE; walrus only lowers tensor ops
                # on DVE/ACT -- Pool compute fails codegen)
                s = vscr.tile([P, d], KDT, tag="vout")
                nc.vector.scalar_tensor_tensor(
                    out=s, in0=qrep[:, b, :], scalar=1.0, in1=qrep[:, b, :],
                    op0=Alu.mult, op1=Alu.mult,
                    accum_out=qss[:, b : b + 1],
                )

            import contextlib

            if loop_n or reps > 1:
                # bench-loop configs: preamble once, incl. qss/mask processing
                # (the tail would otherwise redo these in-place ops every rep)
                emit_qrep(0)
                emit_preamble()
                for b in range(bpc):
                    emit_qnorm(b)
                nc.scalar.activation(out=qss, in_=qss, func=Act.Sqrt)
                nc.vector.tensor_scalar_max(qss, qss, EPS)
                nc.vector.reciprocal(qss, qss)
                nc.scalar.activation(out=maskf, in_=maskf, func=Act.Identity,
                                     bias=negbig[:, 0:1], scale=NEG_BIG)

            loop_cm = tc.For_i(0, loop_n, 1) if loop_n else contextlib.nullcontext()
            with loop_cm:
              for _rep in range(reps):
                # per-rep accumulators (bufs=1 tags -> reps serialize on slots)
                dots = singles.tile([P, cols], F32, tag="dots")
                ssqs = singles.tile([P, cols], F32, tag="ssqs")
                if not do_dve:
                    nc.vector.memset(dots, 0.0)
                if not do_act:
                    nc.vector.memset(ssqs, 1.0)

                # --- main loop: stream keys, fused dot + ssq reductions.
                # DMA emission is software-pipelined `pf` chunks ahead of
                # compute in program order: the Pool sequencer is in-order,
                # so a Pool compute op waiting on chunk data must never sit
                # in front of the SWDGE descriptor-gen for a later chunk.
                # chunk list: (batch, first t-column, n l-tiles).  The first
                # chunk is split into small sub-chunks so the first compute
                # ops aren't stuck behind a full 4 MiB DMA at startup.
                chunks = []
                for b in range(bpc):
                    t0 = 0
                    if b == 0 and cj >= 4:
                        for _ in range(4):
                            chunks.append((b, t0, cj // 4))
                            t0 += cj // 4
                    while t0 < t_cols:
                        chunks.append((b, t0, cj))
                        t0 += cj
                kvs = [
                    keys_ap[b].rearrange("(p t) d -> p (t d)", p=P)
                    for b in range(bpc)
                ]
                tiles = {}
                pf = max(1, kbufs - 2)

                def emit_dma(i):
                    b, t0, nj = chunks[i]
                    kt = kpool.tile([P, nj, d], KDT, tag="kt")
                    kt_flat = kt[:].rearrange("p c d -> p (c d)")
                    src = kvs[b][:, t0 * d : (t0 + nj) * d]
                    if KDT == F32:
                        nc.sync.dma_start(out=kt_flat, in_=src)
                    else:
                        # SWDGE path: f32 -> bf16 cast inside the DMA
                        nc.gpsimd.dma_start(out=kt_flat, in_=src)
                    tiles[i] = kt

                def emit_compute(i):
                    b, t0, nj = chunks[i]
                    kt = tiles.pop(i)
                    pool_dot_i = pool_dot + (pool_dot_alt if i % 2 == 0 else 0)
                    dve_ssq_i = dve_ssq + (
                        dve_ssq_alt if i % dve_ssq_mod == 0 else 0
                    )
                    for j_ in range(nj):
                        idx = b * t_cols + t0 + j_
                        j = (t0 + j_) % cj   # engine split by absolute period
                        if do_dve:
                            if j >= cj - pool_dot_i:
                                pout = vscr.tile([P, d], KDT, tag="pout")
                                nc.gpsimd.scalar_tensor_tensor(
                                    out=pout,
                                    in0=kt[:, j_, :],
                                    scalar=1.0,
                                    in1=qrep[:, b, :],
                                    op0=Alu.mult,
                                    op1=Alu.mult,
                                    accum_out=dots[:, idx : idx + 1],
                                )
                            else:
                                vout = vscr.tile([P, d], KDT, tag="vout")
                                nc.vector.scalar_tensor_tensor(
                                    out=vout,
                                    in0=kt[:, j_, :],
                                    scalar=1.0,
                                    in1=qrep[:, b, :],
                                    op0=Alu.mult,
                                    op1=Alu.mult,
                                    accum_out=dots[:, idx : idx + 1],
                                )
                        if do_act:
                            if j < pool_ssq:
                                pout2 = vscr.tile([P, d], KDT, tag="pout2")
                                nc.gpsimd.scalar_tensor_tensor(
                                    out=pout2,
                                    in0=kt[:, j_, :],
                                    scalar=1.0,
                                    in1=kt[:, j_, :],
                                    op0=Alu.mult,
                                    op1=Alu.mult,
                                    accum_out=ssqs[:, idx : idx + 1],
                                )
                            elif do_dve and j < pool_ssq + dve_ssq_i:
                                vout2 = vscr.tile([P, d], KDT, tag="vout2")
                                nc.vector.scalar_tensor_tensor(
                                    out=vout2,
                                    in0=kt[:, j_, :],
                                    scalar=1.0,
                                    in1=kt[:, j_, :],
                                    op0=Alu.mult,
                                    op1=Alu.mult,
                                    accum_out=ssqs[:, idx : idx + 1],
                                )
                            else:
                                aout = ascr.tile([P, d], KDT, tag="aout")
                                nc.scalar.activation(
                                    out=aout,
                                    in_=kt[:, j_, :],
                                    func=Act.Square,
                                    accum_out=ssqs[:, idx : idx + 1],
                                )

                single = _rep == 0 and not loop_n and reps == 1
                for i in range(len(chunks)):
                    if i == 0 and single:
                        emit_qrep(0)
                    emit_dma(i)
                    if i == min(1, pf - 1) and single:
                        emit_preamble()
                    if i >= pf:
                        emit_compute(i - pf)
                        if single and 1 <= (i - pf) <= bpc:
                            emit_qnorm(i - pf - 1)
                for i in range(len(chunks) - pf, len(chunks)):
                    emit_compute(i)

                # --- batched tail: normalize, mask, softmax, store ---
                den = singles.tile([P, bpc], F32, tag="den")
                if _rep == 0 and not loop_n and reps == 1:
                    nc.scalar.activation(out=qss, in_=qss, func=Act.Sqrt)
                    nc.vector.tensor_scalar_max(qss, qss, EPS)
                    nc.vector.reciprocal(qss, qss)    # 1/||q|| per batch
                nc.scalar.activation(out=ssqs, in_=ssqs, func=Act.Sqrt)  # ||k||
                if not loop_n and reps == 1:
                    # mask -> additive bias {0, -1e30} (deferred from preamble)
                    nc.scalar.activation(out=maskf, in_=maskf, func=Act.Identity,
                                         bias=negbig[:, 0:1], scale=NEG_BIG)
                nc.vector.tensor_scalar_max(ssqs, ssqs, EPS)
                nc.vector.reciprocal(ssqs, ssqs)                         # 1/||k||
                for b in range(bpc):
                    sl = slice(b * t_cols, (b + 1) * t_cols)
                    # fold 1/||q|| into 1/||k||
                    nc.vector.tensor_scalar_mul(ssqs[:, sl], ssqs[:, sl],
                                                qss[:, b : b + 1])
                nc.vector.tensor_mul(dots, dots, ssqs)     # cos scores
                nc.vector.tensor_add(dots, dots, maskf)    # mask bias
                nc.scalar.activation(out=dots, in_=dots, func=Act.Exp)
                # denominator: ones.T @ E sums across partitions; then
                # segmented-reduce the t_cols columns per batch; every
                # partition ends up with the full per-batch sum.
                mm = psum.tile([P, cols], F32, tag="mm")
                nc.tensor.matmul(out=mm, lhsT=ones, rhs=dots,
                                 start=True, stop=True)
                nc.vector.tensor_reduce(
                    out=den,
                    in_=mm[:].rearrange("p (b t) -> p b t", b=bpc),
                    axis=mybir.AxisListType.X,
                    op=Alu.add,
                )
                nc.vector.reciprocal(den, den)
                for b in range(bpc):
                    sl = slice(b * t_cols, (b + 1) * t_cols)
                    nc.vector.tensor_scalar_mul(dots[:, sl], dots[:, sl],
                                                den[:, b : b + 1])
                out_v = out_ap.rearrange("b (p t) -> p b t", p=P)
                nc.sync.dma_start(
                    out=out_v,
                    in_=dots[:].rearrange("p (b t) -> p b t", b=bpc),
                )

    nc.compile()
    return nc


_NC_CACHE = None


def _get_nc():
    global _NC_CACHE
    if _NC_CACHE is None:
        _NC_CACHE = build_nc()
    return _NC_CACHE


def kernel(query: np.ndarray, keys: np.ndarray, mask: np.ndarray) -> np.ndarray:
    assert query.shape == (B, D) and keys.shape == (B, L, D) and mask.shape == (B, L)
    from concourse.bass_utils import run_bass_kernel_spmd

    nc = _get_nc()
    mask_u8 = np.ascontiguousarray(mask).view(np.uint8)
    in_maps = []
    for i in range(N_CORES):
        sl = slice(i * BPC, (i + 1) * BPC)
        in_maps.append(
            {
                "q": np.ascontiguousarray(query[sl], dtype=np.float32),
                "keys": np.ascontiguousarray(keys[sl], dtype=np.float32),
                "mask": np.ascontiguousarray(mask_u8[sl]),
            }
        )
    res = run_bass_kernel_spmd(nc, in_maps, core_ids=list(range(N_CORES)))
    out = np.concatenate([r["out"] for r in res.results], axis=0)
    return out.astype(np.float32, copy=False)


# revision 40
# speedup vs baseline: 1.0432x; 1.0247x over previous
"""CosineAttention Trainium2 Bass kernel.

Computes softmax(cos_sim(keys[b,l,:], query[b,:]) masked) over l, for
B=64, L=4096, D=1024, sharded batch-parallel over 8 NeuronCores
(8 batches per core, 128 MiB of keys per core -> memory bound).

Math per (b, l):
    dot[l]  = sum_d keys[b,l,d] * q[b,d]
    ssq[l]  = sum_d keys[b,l,d]^2
    score   = dot / (max(sqrt(ssq),eps) * max(||q||,eps)) + (mask-1)*1e30
    out     = exp(score) / sum_l exp(score)   (scores in [-1,1]: no max-sub)

Engine plan per core (keys stream at the HBM roofline, compute hides
underneath):
  - DMA  : gpsimd (SWDGE) streams keys in 4 MiB f32 reads, casting to
           bf16 on the fly (2 MiB SBUF tiles); rel err ~4e-4 stays well
           inside the 2e-2 gate while SBUF write pressure halves and
           prefetch depth doubles.
  - DVE  : fused mult+accum (scalar_tensor_tensor) -> dot per l, plus
           1/3 of the ssq ops (load-balances the two engines; both the
           cost model and the HW docs put DVE/ACT reductions at
           1 elem/cycle/lane, so the split is by op count)
  - ACT  : Square with accum_out -> the other 2/3 of ssq (Square lives
           in every ACT table set, so the main loop never reloads
           tables; walrus rejects tensor ops on Pool, so GPSIMD only
           does DMA descriptor work)
  - PE   : ones-matmul for the cross-partition softmax denominator
DMA emission is software-pipelined ahead of compute in program order
(in-order sequencers: a waiting compute op must never sit in front of
descriptor-gen for a later chunk), the first chunk is split into 1 MiB
sub-chunks to shorten the startup ramp, and softmax runs as one
batched tail (2 ACT table loads total instead of 2 per batch).  L is
laid out interleaved: l = p*T + t (p = partition, T = L/128), so keys
loads and the output store are contiguous per partition.
"""

import numpy as np

import concourse.bass as bass
import concourse.tile as tile
from concourse import bacc, mybir

P = 128          # SBUF partitions
B = 64           # full batch
L = 4096
D = 1024
N_CORES = 8
BPC = B // N_CORES   # batches per core
CJ = 8               # l-tiles per DMA chunk (4 MiB f32 reads)

F32 = mybir.dt.float32
BF16 = mybir.dt.bfloat16
U8 = mybir.dt.uint8
Alu = mybir.AluOpType
Act = mybir.ActivationFunctionType

EPS = 1e-12
NEG_BIG = 1.0e30


def build_nc(bpc=BPC, l_dim=L, d=D, cj=CJ, n_cores=N_CORES, reps=1,
             variant="full", kbufs=6, dve_ssq=0, dve_ssq_alt=1, dve_ssq_mod=3,
             pool_ssq=0, pool_dot=0, pool_dot_alt=0, kdt="bf16", loop_n=0,
             epi="tail"):
    do_dve = variant in ("full", "dma_dve")
    do_act = variant in ("full", "dma_act")
    KDT = BF16 if kdt == "bf16" else F32
    t_cols = l_dim // P       # score columns per partition
    nch = t_cols // cj        # chunks per batch
    cols = bpc * t_cols
    assert t_cols * P == l_dim and nch * cj == t_cols

    nc = bacc.Bacc(
        "TRN2",
        target_bir_lowering=False,
        debug=False,
        enable_asserts=False,
        num_devices=n_cores,
    )

    q_t = nc.dram_tensor("q", [bpc, d], F32, kind="ExternalInput")
    keys_t = nc.dram_tensor("keys", [bpc, l_dim, d], F32, kind="ExternalInput")
    mask_t = nc.dram_tensor("mask", [bpc, l_dim], U8, kind="ExternalInput")
    out_t = nc.dram_tensor("out", [bpc, l_dim], F32, kind="ExternalOutput")

    q_ap = q_t.ap()
    keys_ap = keys_t.ap()
    mask_ap = mask_t.ap()
    out_ap = out_t.ap()

    with tile.TileContext(nc) as tc:
        with (
            tc.tile_pool(name="kpool", bufs=kbufs) as kpool,
            tc.tile_pool(name="singles", bufs=1) as singles,
            tc.tile_pool(name="ascr", bufs=2) as ascr,
            tc.tile_pool(name="vscr", bufs=2) as vscr,
            tc.tile_pool(name="psum", bufs=2, space="PSUM") as psum,
        ):
            # --- persistent tiles ---
            qrep = singles.tile([P, bpc, d], KDT)        # q replicated to all partitions
            maskf = singles.tile([P, cols], F32)         # mask -> additive bias
            qss = singles.tile([P, bpc], F32)            # per-batch 1/||q||
            ones = singles.tile([P, P], F32)             # cross-partition sum matmul
            negbig = singles.tile([P, 1], F32)           # bias tile for mask rescale

            nc.vector.memset(ones, 1.0)
            nc.vector.memset(negbig, -NEG_BIG)

            def emit_qrep(b):
                # Broadcast q[b] to all 128 partitions during the DMA
                # (partition step 0), casting to the compute dtype on the
                # fly.  Piecewise per batch: batch 0 goes first so the
                # first dots aren't stuck behind a 4 MiB broadcast.
                q_bcast = bass.AP(
                    tensor=q_ap.tensor,
                    offset=q_ap.offset + b * d,
                    ap=[[0, P], [1, d]],
                )
                nc.gpsimd.dma_start(out=qrep[:, b, :], in_=q_bcast)

            def emit_preamble():
                for b in range(1, bpc):
                    emit_qrep(b)
                # Mask: u8 -> f32 cast during DMA.  DRAM layout per batch is
                # [P, t_cols] with l = p*t_cols + t.
                mask_v = mask_ap.rearrange("b (p t) -> p b t", p=P)
                nc.gpsimd.dma_start(
                    out=maskf[:].rearrange("p (b t) -> p b t", b=bpc), in_=mask_v
                )

                # (mask -> additive-bias rescale is deferred to the tail so
                # ACT's in-order queue isn't blocked on the mask DMA here)

                # (q norms are distributed across the first steady-state
                # chunks -- see emit_qnorm; sqrt/recip happen in the tail)

            def emit_qnorm(b):
                # q norm for one batch (DVE; walrus only lowers tensor ops
                # on DVE/ACT -- Pool compute fails codegen)
                s = vscr.tile([P, d], KDT, tag="vout")
                nc.vector.scalar_tensor_tensor(
                    out=s, in0=qrep[:, b, :], scalar=1.0, in1=qrep[:, b, :],
                    op0=Alu.mult, op1=Alu.mult,
                    accum_out=qss[:, b : b + 1],
                )

            import contextlib

            if loop_n or reps > 1:
                # bench-loop configs: preamble once, incl. qss/mask processing
                # (the tail would otherwise redo these in-place ops every rep)
                emit_qrep(0)
                emit_preamble()
                for b in range(bpc):
                    emit_qnorm(b)
                nc.scalar.activation(out=qss, in_=qss, func=Act.Sqrt)
                nc.vector.tensor_scalar_max(qss, qss, EPS)
                nc.vector.reciprocal(qss, qss)
                nc.scalar.activation(out=maskf, in_=maskf, func=Act.Identity,
                                     bias=negbig[:, 0:1], scale=NEG_BIG)

            loop_cm = tc.For_i(0, loop_n, 1) if loop_n else contextlib.nullcontext()
            with loop_cm:
              for _rep in range(reps):
                # per-rep accumulators (bufs=1 tags -> reps serialize on slots)
                dots = singles.tile([P, cols], F32, tag="dots")
                ssqs = singles.tile([P, cols], F32, tag="ssqs")
                if not do_dve:
                    nc.vector.memset(dots, 0.0)
                if not do_act:
                    nc.vector.memset(ssqs, 1.0)

                # --- main loop: stream keys, fused dot + ssq reductions.
                # DMA emission is software-pipelined `pf` chunks ahead of
                # compute in program order: the Pool sequencer is in-order,
                # so a Pool compute op waiting on chunk data must never sit
                # in front of the SWDGE descriptor-gen for a later chunk.
                # chunk list: (batch, first t-column, n l-tiles).  The first
                # chunk is split into small sub-chunks so the first compute
                # ops aren't stuck behind a full 4 MiB DMA at startup.
                chunks = []
                for b in range(bpc):
                    t0 = 0
                    if b == 0 and cj >= 4:
                        for _ in range(4):
                            chunks.append((b, t0, cj // 4))
                            t0 += cj // 4
                    while t0 < t_cols:
                        chunks.append((b, t0, cj))
                        t0 += cj
                kvs = [
                    keys_ap[b].rearrange("(p t) d -> p (t d)", p=P)
                    for b in range(bpc)
                ]
                tiles = {}
                pf = max(1, kbufs - 2)

                def emit_dma(i):
                    b, t0, nj = chunks[i]
                    kt = kpool.tile([P, nj, d], KDT, tag="kt")
                    kt_flat = kt[:].rearrange("p c d -> p (c d)")
                    src = kvs[b][:, t0 * d : (t0 + nj) * d]
                    if KDT == F32:
                        nc.sync.dma_start(out=kt_flat, in_=src)
                    else:
                        # SWDGE path: f32 -> bf16 cast inside the DMA
                        nc.gpsimd.dma_start(out=kt_flat, in_=src)
                    tiles[i] = kt

                def emit_compute(i):
                    b, t0, nj = chunks[i]
                    kt = tiles.pop(i)
                    pool_dot_i = pool_dot + (pool_dot_alt if i % 2 == 0 else 0)
                    dve_ssq_i = dve_ssq + (
                        dve_ssq_alt if i % dve_ssq_mod == 0 else 0
                    )
                    for j_ in range(nj):
                        idx = b * t_cols + t0 + j_
                        j = (t0 + j_) % cj   # engine split by absolute period
                        if do_dve:
                            if j >= cj - pool_dot_i:
                                pout = vscr.tile([P, d], KDT, tag="pout")
                                nc.gpsimd.scalar_tensor_tensor(
                                    out=pout,
                                    in0=kt[:, j_, :],
                                    scalar=1.0,
                                    in1=qrep[:, b, :],
                                    op0=Alu.mult,
                                    op1=Alu.mult,
                                    accum_out=dots[:, idx : idx + 1],
                                )
                            else:
                                vout = vscr.tile([P, d], KDT, tag="vout")
                                nc.vector.scalar_tensor_tensor(
                                    out=vout,
                                    in0=kt[:, j_, :],
                                    scalar=1.0,
                                    in1=qrep[:, b, :],
                                    op0=Alu.mult,
                                    op1=Alu.mult,
                                    accum_out=dots[:, idx : idx + 1],
                                )
                        if do_act:
                            if j < pool_ssq:
                                pout2 = vscr.tile([P, d], KDT, tag="pout2")
                                nc.gpsimd.scalar_tensor_tensor(
                                    out=pout2,
                                    in0=kt[:, j_, :],
                                    scalar=1.0,
                                    in1=kt[:, j_, :],
                                    op0=Alu.mult,
                                    op1=Alu.mult,
                                    accum_out=ssqs[:, idx : idx + 1],
                                )
                            elif do_dve and j < pool_ssq + dve_ssq_i:
                                vout2 = vscr.tile([P, d], KDT, tag="vout2")
                                nc.vector.scalar_tensor_tensor(
                                    out=vout2,
                                    in0=kt[:, j_, :],
                                    scalar=1.0,
                                    in1=kt[:, j_, :],
                                    op0=Alu.mult,
                                    op1=Alu.mult,
                                    accum_out=ssqs[:, idx : idx + 1],
                                )
                            else:
                                aout = ascr.tile([P, d], KDT, tag="aout")
                                nc.scalar.activation(
                                    out=aout,
                                    in_=kt[:, j_, :],
                                    func=Act.Square,
                                    accum_out=ssqs[:, idx : idx + 1],
                                )

                single = _rep == 0 and not loop_n and reps == 1
                for i in range(len(chunks)):
                    if i == 0 and single:
                        emit_qrep(0)
                    emit_dma(i)
                    if i == min(1, pf - 1) and single:
                        emit_preamble()
                    if i >= pf:
                        emit_compute(i - pf)
                        if single and 1 <= (i - pf) <= bpc:
                            emit_qnorm(i - pf - 1)
                for i in range(len(chunks) - pf, len(chunks)):
                    emit_compute(i)

                # --- batched tail: normalize, mask, softmax, store ---
                den = singles.tile([P, bpc], F32, tag="den")
                if _rep == 0 and not loop_n and reps == 1:
                    nc.scalar.activation(out=qss, in_=qss, func=Act.Sqrt)
                    nc.vector.tensor_scalar_max(qss, qss, EPS)
                    nc.vector.reciprocal(qss, qss)    # 1/||q|| per batch
                nc.scalar.activation(out=ssqs, in_=ssqs, func=Act.Sqrt)  # ||k||
                if not loop_n and reps == 1:
                    # mask -> additive bias {0, -1e30} (deferred from preamble)
                    nc.scalar.activation(out=maskf, in_=maskf, func=Act.Identity,
                                         bias=negbig[:, 0:1], scale=NEG_BIG)
                nc.vector.tensor_scalar_max(ssqs, ssqs, EPS)
                nc.vector.reciprocal(ssqs, ssqs)                         # 1/||k||
                for b in range(bpc):
                    sl = slice(b * t_cols, (b + 1) * t_cols)
                    # fold 1/||q|| into 1/||k||
                    nc.vector.tensor_scalar_mul(ssqs[:, sl], ssqs[:, sl],
                                                qss[:, b : b + 1])
                nc.vector.tensor_mul(dots, dots, ssqs)     # cos scores
                nc.vector.tensor_add(dots, dots, maskf)    # mask bias
                nc.scalar.activation(out=dots, in_=dots, func=Act.Exp)
                # denominator: ones.T @ E sums across partitions; then
                # segmented-reduce the t_cols columns per batch; every
                # partition ends up with the full per-batch sum.
                mm = psum.tile([P, cols], F32, tag="mm")
                nc.tensor.matmul(out=mm, lhsT=ones, rhs=dots,
                                 start=True, stop=True)
                nc.vector.tensor_reduce(
                    out=den,
                    in_=mm[:].rearrange("p (b t) -> p b t", b=bpc),
                    axis=mybir.AxisListType.X,
                    op=Alu.add,
                )
                nc.vector.reciprocal(den, den)
                for b in range(bpc):
                    sl = slice(b * t_cols, (b + 1) * t_cols)
                    nc.vector.tensor_scalar_mul(dots[:, sl], dots[:, sl],
                                                den[:, b : b + 1])
                out_v = out_ap.rearrange("b (p t) -> p b t", p=P)
                nc.sync.dma_start(
                    out=out_v,
                    in_=dots[:].rearrange("p (b t) -> p b t", b=bpc),
                )

    nc.compile()
    return nc


_NC_CACHE = None


def _get_nc():
    global _NC_CACHE
    if _NC_CACHE is None:
        _NC_CACHE = build_nc()
    return _NC_CACHE


def kernel(query: np.ndarray, keys: np.ndarray, mask: np.ndarray) -> np.ndarray:
    assert query.shape == (B, D) and keys.shape == (B, L, D) and mask.shape == (B, L)
    from concourse.bass_utils import run_bass_kernel_spmd

    nc = _get_nc()
    mask_u8 = np.ascontiguousarray(mask).view(np.uint8)
    in_maps = []
    for i in range(N_CORES):
        sl = slice(i * BPC, (i + 1) * BPC)
        in_maps.append(
            {
                "q": np.ascontiguousarray(query[sl], dtype=np.float32),
                "keys": np.ascontiguousarray(keys[sl], dtype=np.float32),
                "mask": np.ascontiguousarray(mask_u8[sl]),
            }
        )
    res = run_bass_kernel_spmd(nc, in_maps, core_ids=list(range(N_CORES)))
    out = np.concatenate([r["out"] for r in res.results], axis=0)
    return out.astype(np.float32, copy=False)
